# revision 39
# baseline (speedup 1.0000x reference)
"""Trainium2 Bass kernel for LoRACrossAttnProcessor (v2, bf16).

Strategy:
- Host: fold LoRA (W_eff = W + up @ down, exact in f64), pre-transpose
  X/E/W so every device matmul contracts over the partition dim, cast
  everything to bf16 (rel err ~0.4% << 2e-2 tolerance). Wv is scattered
  into a (head, tile)-pair layout with zero padding so attention-output
  matmuls can accumulate whole 128-row PSUM tiles (no partition-shift
  DMAs).
- Shard: data-parallel over batch, 2 batch items per core, 8 cores.
- Device (per core; PSUM fp32, SBUF bf16):
    K.T tiles  = Wk_eff.T-blocks @ E.T   -> parity-masked kte/kto
    V          = E @ Wvm (pair layout)   -> vm [77, 16*128]
    Q.T        = Wq_eff.T-blocks @ X.T   -> qt [128, 10, 1024]
    per (b, st, h): scores.T = kt_h.T @ qt  [77, 512]
      exps = exp(scores.T * scale) (ACT), sumexp = ones.T @ exps (PE),
      recip (DVE), partition-broadcast (GPSIMD), expn = exps * bc (DVE)
    per (b, st, tile): A.T tile = sum_h vm_pair @ expn_h  (accumulated
      full-128-row matmuls; zeros in vm kill other heads' rows)
    O[s, c]    = A.T-blocks.T @ Wo_eff.T  (natural layout; contiguous
      2.5KB-row DMA to DRAM -- the v1 kernel's transposed output DMA
      was 2.6M 4-byte descriptors and 90% of its runtime)
- Host: gather batches, cast fp32, add bo.
"""

import numpy as np
from contextlib import ExitStack

import ml_dtypes

import concourse.bass as bass
import concourse.mybir as mybir
import concourse.tile as tile
from concourse import bacc, bass_isa
from concourse.bass_utils import run_bass_kernel_spmd

F32 = mybir.dt.float32
BF16 = mybir.dt.bfloat16
F8 = mybir.dt.float8e4
AF = mybir.ActivationFunctionType
MULT = mybir.AluOpType.mult

H = 8
B, S, C = 16, 1024, 1280
SENC, CENC = 77, 1024
D = C // H  # 160
NCORES = 8
BPC = B // NCORES  # 2 batches per core
P = 128
NCI_Q = C // P  # 10 contraction tiles for Q/O proj
NCI_KV = CENC // P  # 8 contraction tiles for K/V proj
NCO = C // P  # 10 output-channel tiles
NST = S // 512  # 2 seq chunks of 512
SENC2 = 2 * SENC  # 154
ATTN_SCALE = 1.0 / float(np.sqrt(D))
OCHUNKS = [(0, 512), (512, 512), (1024, 256)]

# (head, tile) pairs: head h covers channels [160h, 160h+160); tile t covers
# [128t, 128t+128). Each pair gets one 128-col slot in the vm layout.
PAIRS = []
for _h in range(H):
    for _t in range(NCO):
        lo = max(D * _h, P * _t)
        hi = min(D * _h + D, P * _t + P)
        if lo < hi:
            PAIRS.append((_h, _t, lo, hi))
NPAIR = len(PAIRS)  # 16
PAIRS_OF_TILE = {t: [i for i, p in enumerate(PAIRS) if p[1] == t] for t in range(NCO)}
TILES_OF_HEAD = {h: sorted({p[1] for p in PAIRS if p[0] == h}) for h in range(H)}


def aligned_ranges(r0, r1):
    """Decompose [r0, r1) (within one 128 tile) into blocks of size 32/64/128
    with offset % size == 0 (SBUF partition-access alignment rule)."""
    out = []
    g = r0
    while g < r1:
        s = 128
        while s > r1 - g or g % s != 0:
            s //= 2
        out.append((g, s))
        g += s
    return out


def build():
    nc = bacc.Bacc("TRN2", target_bir_lowering=False, debug=False)
    xt_d = nc.dram_tensor("xt", [BPC, C, S], BF16, kind="ExternalInput")
    et_d = nc.dram_tensor("et", [CENC, SENC2], BF16, kind="ExternalInput")
    wqt_d = nc.dram_tensor("wqt", [C, C], BF16, kind="ExternalInput")
    wkt_d = nc.dram_tensor("wkt", [CENC, C], BF16, kind="ExternalInput")
    wvm_d = nc.dram_tensor("wvm", [CENC, NPAIR * P], BF16, kind="ExternalInput")
    wot_d = nc.dram_tensor("wot", [C, C], BF16, kind="ExternalInput")
    out_d = nc.dram_tensor("out", [BPC, S, C], BF16, kind="ExternalOutput")

    with tile.TileContext(nc) as tc, ExitStack() as ctx:
        big = ctx.enter_context(tc.tile_pool(name="big", bufs=4))
        wpool = ctx.enter_context(tc.tile_pool(name="wpool", bufs=1))
        persist = ctx.enter_context(tc.tile_pool(name="persist", bufs=1))
        expnp = ctx.enter_context(tc.tile_pool(name="expnp", bufs=2))
        divp = ctx.enter_context(tc.tile_pool(name="divp", bufs=8))
        smallp = ctx.enter_context(tc.tile_pool(name="smallp", bufs=2))
        stag = ctx.enter_context(tc.tile_pool(name="stag", bufs=1))
        psA = ctx.enter_context(tc.tile_pool(name="psA", bufs=3, space="PSUM"))
        psSE = ctx.enter_context(tc.tile_pool(name="psSE", bufs=3, space="PSUM"))
        psV = ctx.enter_context(tc.tile_pool(name="psV", bufs=2, space="PSUM"))

        # ---- load E.T  [1024, 154] -> et_s [128, 8, 154] ----
        et_s = persist.tile([P, NCI_KV, SENC2], BF16, tag="et")
        nc.sync.dma_start(
            out=et_s, in_=et_d.ap().rearrange("(ci p) e -> p ci e", p=P)
        )

        # ---- constants ----
        ones77 = persist.tile([SENC, 1], BF16, tag="ones77")
        nc.vector.memset(ones77, 1.0)

        # ---- early DMAs, queue-ordered by first use: Wk, Wq, then X.T ----
        # wk and wo share slot wA (wo DMA fires once K-proj drains); wq gets
        # wB so it prefetches in parallel; wvm gets its own wC.
        wk_s = wpool.tile([P, NCI_KV, C], BF16, tag="wA")
        nc.sync.dma_start(
            out=wk_s, in_=wkt_d.ap().rearrange("(ci p) c -> p ci c", p=P)
        )
        wq_s = wpool.tile([P, NCI_Q, C], BF16, tag="wB", name="wq_s")
        for ch in range(0, C, 640):
            nc.sync.dma_start(
                out=wq_s[:, :, ch : ch + 640],
                in_=wqt_d.ap()[:, ch : ch + 640].rearrange(
                    "(ci p) c -> p ci c", p=P
                ),
            )
        x_s = [None] * BPC
        for b in range(BPC):
            x_s[b] = big.tile([P, NCI_Q, S], BF16, tag="big", name=f"xt{b}")
            for st in range(NST):
                sl = slice(st * 512, st * 512 + 512)
                nc.sync.dma_start(
                    out=x_s[b][:, :, sl],
                    in_=xt_d.ap()[b, :, sl].rearrange("(ci p) s -> p ci s", p=P),
                )

        # ---- K.T projection: kte/kto[t] = parity-masked [128, 154] ----
        kt_r = []
        for t in range(NCO):
            ps = psA.tile([P, 512], F32, tag="ps")
            for ci in range(NCI_KV):
                nc.tensor.matmul(
                    ps[:, :SENC2],
                    wk_s[:, ci, t * P : (t + 1) * P],
                    et_s[:, ci, :],
                    start=(ci == 0),
                    stop=(ci == NCI_KV - 1),
                )
            kte = persist.tile([P, SENC2], BF16, tag=f"kte{t}", name=f"kte{t}")
            kto = persist.tile([P, SENC2], BF16, tag=f"kto{t}", name=f"kto{t}")
            nc.vector.memset(kte, 0.0)
            nc.vector.memset(kto, 0.0)
            for h in range(H):
                r0 = max(D * h, P * t)
                r1 = min(D * h + D, P * t + P)
                if r0 >= r1:
                    continue
                dst = kte if h % 2 == 0 else kto
                for o, sz in aligned_ranges(r0 - P * t, r1 - P * t):
                    nc.vector.tensor_copy(
                        out=dst[o : o + sz, :], in_=ps[o : o + sz, :SENC2]
                    )
            kt_r.append((kte, kto))

        qt = [None] * BPC
        at = [None] * BPC
        vm = [None] * BPC

        def v_proj():
            # V projection into (head,tile)-pair layout: vm[b] [77, 16*128]
            wvm_s = wpool.tile([P, NCI_KV, NPAIR * P], BF16, tag="wC", name="wvm_s")
            nc.sync.dma_start(
                out=wvm_s, in_=wvm_d.ap().rearrange("(ci p) c -> p ci c", p=P)
            )
            for b in range(BPC):
                vm[b] = persist.tile(
                    [SENC, NPAIR, P], BF16, tag=f"vm{b}", name=f"vm{b}"
                )
            for b in range(BPC):
                for cc in range(0, NPAIR * P, 512):
                    ps = psA.tile([P, 512], F32, tag="ps")
                    for ci in range(NCI_KV):
                        nc.tensor.matmul(
                            ps[:SENC, :],
                            et_s[:, ci, b * SENC : (b + 1) * SENC],
                            wvm_s[:, ci, cc : cc + 512],
                            start=(ci == 0),
                            stop=(ci == NCI_KV - 1),
                        )
                    nc.vector.tensor_copy(
                        out=vm[b][:, cc // P : cc // P + 4, :].rearrange(
                            "p a b -> p (a b)"
                        ),
                        in_=ps[:SENC, :],
                    )

        def q_proj(b):
            qt[b] = big.tile([P, NCO, S], BF16, tag="big", name=f"qt{b}")
            for st in range(NST):
                for co in range(NCO):
                    sl = slice(st * 512, st * 512 + 512)
                    ps = psA.tile([P, 512], F32, tag="ps")
                    for ci in range(NCI_Q):
                        nc.tensor.matmul(
                            ps,
                            wq_s[:, ci, co * P : (co + 1) * P],
                            x_s[b][:, ci, sl],
                            start=(ci == 0),
                            stop=(ci == NCI_Q - 1),
                        )
                    nc.scalar.copy(out=qt[b][:, co, sl], in_=ps)

        def attention(b):
            at[b] = big.tile([P, NCO, S], BF16, tag="big", name=f"at{b}")
            bsl = slice(b * SENC, (b + 1) * SENC)
            for st in range(NST):
                sl = slice(st * 512, st * 512 + 512)
                exps = expnp.tile([SENC, H, 512], BF16, tag="expn")
                divh = [None] * H
                for h in range(H):
                    tiles = TILES_OF_HEAD[h]
                    ps_s = psSE.tile([SENC, 512], F32, tag="ps")
                    for i, t in enumerate(tiles):
                        nc.tensor.matmul(
                            ps_s,
                            kt_r[t][h % 2][:, bsl],
                            qt[b][:, t, sl],
                            start=(i == 0),
                            stop=(i == len(tiles) - 1),
                        )
                    nc.scalar.activation(
                        out=exps[:, h, :], in_=ps_s, func=AF.Exp, scale=ATTN_SCALE
                    )
                    ps_se = psSE.tile([SENC, 512], F32, tag="ps", name="ps_se")
                    nc.tensor.matmul(
                        ps_se[0:1, :], ones77, exps[:, h, :], start=True, stop=True
                    )
                    nc.vector.reciprocal_approx_fast(
                        out=ps_se[0:1, :], in_=ps_se[0:1, :]
                    )
                    recb = smallp.tile([1, 512], BF16, tag="recb", bufs=2)
                    nc.vector.tensor_copy(out=recb, in_=ps_se[0:1, :])
                    divh[h] = divp.tile([P, 512], BF16, tag="divh", name=f"divh{h}")
                    nc.gpsimd.partition_broadcast(divh[h], recb)
                for t in range(NCO):
                    pairs = PAIRS_OF_TILE[t]
                    ps_av = psV.tile([P, 512], F32, tag="ps")
                    for i, pi in enumerate(pairs):
                        ph = PAIRS[pi][0]
                        nc.tensor.matmul(
                            ps_av,
                            vm[b][:, pi, :],
                            exps[:, ph, :],
                            start=(i == 0),
                            stop=(i == len(pairs) - 1),
                        )
                    # evacuate with fused 1/Z: at = ps_av * divh[head(row)]
                    for pi in pairs:
                        ph, _, lo, hi = PAIRS[pi]
                        for o, sz in aligned_ranges(lo - P * t, hi - P * t):
                            nc.vector.scalar_tensor_tensor(
                                out=at[b][o : o + sz, t, sl],
                                in0=ps_av[o : o + sz, :],
                                scalar=1.0,
                                in1=divh[ph][o : o + sz, :],
                                op0=MULT,
                                op1=MULT,
                            )

        def o_proj(b, wo_s):
            for stile in range(S // P):
                s0 = stile * P
                ost = stag.tile([P, C], BF16, tag="ost")
                pso = [
                    psA.tile([P, 512], F32, tag="ps", name=f"pso{k}")
                    for k in range(len(OCHUNKS))
                ]
                for ci in range(NCI_Q):
                    for k, (c0, cn) in enumerate(OCHUNKS):
                        nc.tensor.matmul(
                            pso[k][:, :cn],
                            at[b][:, ci, s0 : s0 + P],
                            wo_s[:, ci, c0 : c0 + cn],
                            start=(ci == 0),
                            stop=(ci == NCI_Q - 1),
                        )
                for k, (c0, cn) in enumerate(OCHUNKS):
                    nc.vector.tensor_copy(
                        out=ost[:, c0 : c0 + cn], in_=pso[k][:, :cn]
                    )
                nc.sync.dma_start(out=out_d.ap()[b, s0 : s0 + P, :], in_=ost)

        # b-major interleave: attention(b0) overlaps q_proj(b1) on PE;
        # o_proj(b0) overlaps attention(b1).
        q_proj(0)
        v_proj()
        wo_s = wpool.tile([P, NCI_Q, C], BF16, tag="wA", name="wo_s")
        nc.sync.dma_start(
            out=wo_s, in_=wot_d.ap().rearrange("(ci p) c -> p ci c", p=P)
        )
        attention(0)
        q_proj(1)
        attention(1)
        o_proj(0, wo_s)
        o_proj(1, wo_s)

    nc.compile()
    return nc


_NC_CACHE = []


def _get_nc():
    if not _NC_CACHE:
        _NC_CACHE.append(build())
    return _NC_CACHE[0]


def make_in_maps(hidden_states, encoder_hidden_states, Wq, Wk, Wv, Wo,
                 q_down, q_up, k_down, k_up, v_down, v_up, o_down, o_up):
    bf = ml_dtypes.bfloat16
    f8 = ml_dtypes.float8_e4m3
    wq = (Wq.astype(np.float64) + q_up.astype(np.float64) @ q_down.astype(np.float64))
    wk = (Wk.astype(np.float64) + k_up.astype(np.float64) @ k_down.astype(np.float64))
    wv = (Wv.astype(np.float64) + v_up.astype(np.float64) @ v_down.astype(np.float64))
    wo = (Wo.astype(np.float64) + o_up.astype(np.float64) @ o_down.astype(np.float64))
    wqt = np.ascontiguousarray(wq.T).astype(bf)
    wkt = np.ascontiguousarray(wk.T).astype(bf)
    wot = np.ascontiguousarray(wo.T).astype(bf)
    wvt = wv.T  # [CENC, C] f64
    wvm = np.zeros((CENC, NPAIR * P), np.float64)
    for i, (h, t, lo, hi) in enumerate(PAIRS):
        wvm[:, i * P + (lo - P * t) : i * P + (hi - P * t)] = wvt[:, lo:hi]
    wvm = wvm.astype(bf)

    in_maps = []
    for c in range(NCORES):
        hs = hidden_states[c * BPC : (c + 1) * BPC]  # [2, S, C]
        xt = np.ascontiguousarray(hs.transpose(0, 2, 1)).astype(bf)
        enc = encoder_hidden_states[c * BPC : (c + 1) * BPC]  # [2, 77, 1024]
        et = np.empty((CENC, SENC2), np.float32)
        for b in range(BPC):
            et[:, b * SENC : (b + 1) * SENC] = enc[b].T
        in_maps.append(
            {
                "xt": xt,
                "et": et.astype(bf),
                "wqt": wqt,
                "wkt": wkt,
                "wvm": wvm,
                "wot": wot,
            }
        )
    return in_maps


def kernel(hidden_states, encoder_hidden_states, Wq, Wk, Wv, Wo, bo,
           q_down, q_up, k_down, k_up, v_down, v_up, o_down, o_up):
    nc = _get_nc()
    in_maps = make_in_maps(
        hidden_states, encoder_hidden_states, Wq, Wk, Wv, Wo,
        q_down, q_up, k_down, k_up, v_down, v_up, o_down, o_up,
    )
    res = run_bass_kernel_spmd(nc, in_maps, list(range(NCORES)))
    out = np.concatenate(
        [np.asarray(res.results[c]["out"]).astype(np.float32) for c in range(NCORES)],
        axis=0,
    )
    out = out + bo.astype(np.float32)[None, None, :]
    return out.astype(np.float32)


# revision 43
# speedup vs baseline: 1.0289x; 1.0289x over previous
"""Trainium2 Bass kernel for LoRACrossAttnProcessor (v2, bf16).

Strategy:
- Host: fold LoRA (W_eff = W + up @ down, exact in f64), pre-transpose
  X/E/W so every device matmul contracts over the partition dim, cast
  everything to bf16 (rel err ~0.4% << 2e-2 tolerance). Wv is scattered
  into a (head, tile)-pair layout with zero padding so attention-output
  matmuls can accumulate whole 128-row PSUM tiles (no partition-shift
  DMAs).
- Shard: data-parallel over batch, 2 batch items per core, 8 cores.
- Device (per core; PSUM fp32, SBUF bf16):
    K.T tiles  = Wk_eff.T-blocks @ E.T   -> parity-masked kte/kto
    V          = E @ Wvm (pair layout)   -> vm [77, 16*128]
    Q.T        = Wq_eff.T-blocks @ X.T   -> qt [128, 10, 1024]
    per (b, st, h): scores.T = kt_h.T @ qt  [77, 512]
      exps = exp(scores.T * scale) (ACT), sumexp = ones.T @ exps (PE),
      recip (DVE), partition-broadcast (GPSIMD), expn = exps * bc (DVE)
    per (b, st, tile): A.T tile = sum_h vm_pair @ expn_h  (accumulated
      full-128-row matmuls; zeros in vm kill other heads' rows)
    O[s, c]    = A.T-blocks.T @ Wo_eff.T  (natural layout; contiguous
      2.5KB-row DMA to DRAM -- the v1 kernel's transposed output DMA
      was 2.6M 4-byte descriptors and 90% of its runtime)
- Host: gather batches, cast fp32, add bo.
"""

import numpy as np
from contextlib import ExitStack

import ml_dtypes

import concourse.bass as bass
import concourse.mybir as mybir
import concourse.tile as tile
from concourse import bacc, bass_isa
from concourse.bass_utils import run_bass_kernel_spmd

F32 = mybir.dt.float32
BF16 = mybir.dt.bfloat16
F8 = mybir.dt.float8e4
AF = mybir.ActivationFunctionType
MULT = mybir.AluOpType.mult

H = 8
B, S, C = 16, 1024, 1280
SENC, CENC = 77, 1024
D = C // H  # 160
NCORES = 8
BPC = B // NCORES  # 2 batches per core
P = 128
NCI_Q = C // P  # 10 contraction tiles for Q/O proj
NCI_KV = CENC // P  # 8 contraction tiles for K/V proj
NCO = C // P  # 10 output-channel tiles
NST = S // 512  # 2 seq chunks of 512
SENC2 = 2 * SENC  # 154
ATTN_SCALE = 1.0 / float(np.sqrt(D))
OCHUNKS = [(0, 512), (512, 512), (1024, 256)]

# (head, tile) pairs: head h covers channels [160h, 160h+160); tile t covers
# [128t, 128t+128). Each pair gets one 128-col slot in the vm layout.
PAIRS = []
for _h in range(H):
    for _t in range(NCO):
        lo = max(D * _h, P * _t)
        hi = min(D * _h + D, P * _t + P)
        if lo < hi:
            PAIRS.append((_h, _t, lo, hi))
NPAIR = len(PAIRS)  # 16
PAIRS_OF_TILE = {t: [i for i, p in enumerate(PAIRS) if p[1] == t] for t in range(NCO)}
TILES_OF_HEAD = {h: sorted({p[1] for p in PAIRS if p[0] == h}) for h in range(H)}


def aligned_ranges(r0, r1):
    """Decompose [r0, r1) (within one 128 tile) into blocks of size 32/64/128
    with offset % size == 0 (SBUF partition-access alignment rule)."""
    out = []
    g = r0
    while g < r1:
        s = 128
        while s > r1 - g or g % s != 0:
            s //= 2
        out.append((g, s))
        g += s
    return out


def build():
    nc = bacc.Bacc("TRN2", target_bir_lowering=False, debug=False)
    xt_d = nc.dram_tensor("xt", [BPC, C, S], BF16, kind="ExternalInput")
    et_d = nc.dram_tensor("et", [CENC, SENC2], BF16, kind="ExternalInput")
    wqt_d = nc.dram_tensor("wqt", [C, C], BF16, kind="ExternalInput")
    wkt_d = nc.dram_tensor("wkt", [CENC, C], BF16, kind="ExternalInput")
    wvm_d = nc.dram_tensor("wvm", [CENC, NPAIR * P], BF16, kind="ExternalInput")
    wot_d = nc.dram_tensor("wot", [C, C], BF16, kind="ExternalInput")
    out_d = nc.dram_tensor("out", [BPC, S, C], BF16, kind="ExternalOutput")

    with tile.TileContext(nc) as tc, ExitStack() as ctx:
        big = ctx.enter_context(tc.tile_pool(name="big", bufs=4))
        wpool = ctx.enter_context(tc.tile_pool(name="wpool", bufs=1))
        persist = ctx.enter_context(tc.tile_pool(name="persist", bufs=1))
        expnp = ctx.enter_context(tc.tile_pool(name="expnp", bufs=2))
        divp = ctx.enter_context(tc.tile_pool(name="divp", bufs=7))
        smallp = ctx.enter_context(tc.tile_pool(name="smallp", bufs=2))
        stag = ctx.enter_context(tc.tile_pool(name="stag", bufs=2))
        psA = ctx.enter_context(tc.tile_pool(name="psA", bufs=3, space="PSUM"))
        psSE = ctx.enter_context(tc.tile_pool(name="psSE", bufs=3, space="PSUM"))
        psV = ctx.enter_context(tc.tile_pool(name="psV", bufs=2, space="PSUM"))

        # ---- load E.T  [1024, 154] -> et_s [128, 8, 154] ----
        et_s = persist.tile([P, NCI_KV, SENC2], BF16, tag="et")
        nc.sync.dma_start(
            out=et_s, in_=et_d.ap().rearrange("(ci p) e -> p ci e", p=P)
        )

        # ---- constants ----
        ones77 = persist.tile([SENC, 1], BF16, tag="ones77")
        nc.vector.memset(ones77, 1.0)

        # ---- early DMAs, queue-ordered by first use: Wk, Wq, then X.T ----
        # wk and wo share slot wA (wo DMA fires once K-proj drains); wq gets
        # wB so it prefetches in parallel; wvm gets its own wC.
        wk_s = wpool.tile([P, NCI_KV, C], BF16, tag="wA")
        nc.sync.dma_start(
            out=wk_s, in_=wkt_d.ap().rearrange("(ci p) c -> p ci c", p=P)
        )
        wq_s = wpool.tile([P, NCI_Q, C], BF16, tag="wB", name="wq_s")
        for ch in range(0, C, 640):
            nc.sync.dma_start(
                out=wq_s[:, :, ch : ch + 640],
                in_=wqt_d.ap()[:, ch : ch + 640].rearrange(
                    "(ci p) c -> p ci c", p=P
                ),
            )
        x_s = [None] * BPC
        for b in range(BPC):
            x_s[b] = big.tile([P, NCI_Q, S], BF16, tag="big", name=f"xt{b}")
            for st in range(NST):
                sl = slice(st * 512, st * 512 + 512)
                nc.sync.dma_start(
                    out=x_s[b][:, :, sl],
                    in_=xt_d.ap()[b, :, sl].rearrange("(ci p) s -> p ci s", p=P),
                )

        # ---- K.T projection: kte/kto[t] = parity-masked [128, 154] ----
        kt_r = []
        for t in range(NCO):
            ps = psA.tile([P, 512], F32, tag="ps")
            for ci in range(NCI_KV):
                nc.tensor.matmul(
                    ps[:, :SENC2],
                    wk_s[:, ci, t * P : (t + 1) * P],
                    et_s[:, ci, :],
                    start=(ci == 0),
                    stop=(ci == NCI_KV - 1),
                )
            kte = persist.tile([P, SENC2], BF16, tag=f"kte{t}", name=f"kte{t}")
            kto = persist.tile([P, SENC2], BF16, tag=f"kto{t}", name=f"kto{t}")
            nc.vector.memset(kte, 0.0)
            nc.vector.memset(kto, 0.0)
            for h in range(H):
                r0 = max(D * h, P * t)
                r1 = min(D * h + D, P * t + P)
                if r0 >= r1:
                    continue
                dst = kte if h % 2 == 0 else kto
                for o, sz in aligned_ranges(r0 - P * t, r1 - P * t):
                    nc.vector.tensor_copy(
                        out=dst[o : o + sz, :], in_=ps[o : o + sz, :SENC2]
                    )
            kt_r.append((kte, kto))

        qt = [None] * BPC
        at = [None] * BPC
        vm = [None] * BPC

        def v_proj():
            # V projection into (head,tile)-pair layout: vm[b] [77, 16*128]
            wvm_s = wpool.tile([P, NCI_KV, NPAIR * P], BF16, tag="wC", name="wvm_s")
            nc.sync.dma_start(
                out=wvm_s, in_=wvm_d.ap().rearrange("(ci p) c -> p ci c", p=P)
            )
            for b in range(BPC):
                vm[b] = persist.tile(
                    [SENC, NPAIR, P], BF16, tag=f"vm{b}", name=f"vm{b}"
                )
            for b in range(BPC):
                for cc in range(0, NPAIR * P, 512):
                    ps = psA.tile([P, 512], F32, tag="ps")
                    for ci in range(NCI_KV):
                        nc.tensor.matmul(
                            ps[:SENC, :],
                            et_s[:, ci, b * SENC : (b + 1) * SENC],
                            wvm_s[:, ci, cc : cc + 512],
                            start=(ci == 0),
                            stop=(ci == NCI_KV - 1),
                        )
                    nc.vector.tensor_copy(
                        out=vm[b][:, cc // P : cc // P + 4, :].rearrange(
                            "p a b -> p (a b)"
                        ),
                        in_=ps[:SENC, :],
                    )

        def q_proj(b):
            qt[b] = big.tile([P, NCO, S], BF16, tag="big", name=f"qt{b}")
            for st in range(NST):
                for co in range(NCO):
                    sl = slice(st * 512, st * 512 + 512)
                    ps = psA.tile([P, 512], F32, tag="ps")
                    for ci in range(NCI_Q):
                        nc.tensor.matmul(
                            ps,
                            wq_s[:, ci, co * P : (co + 1) * P],
                            x_s[b][:, ci, sl],
                            start=(ci == 0),
                            stop=(ci == NCI_Q - 1),
                        )
                    nc.scalar.copy(out=qt[b][:, co, sl], in_=ps)

        def attention(b):
            at[b] = big.tile([P, NCO, S], BF16, tag="big", name=f"at{b}")
            bsl = slice(b * SENC, (b + 1) * SENC)
            for st in range(NST):
                sl = slice(st * 512, st * 512 + 512)
                exps = expnp.tile([SENC, H, 512], BF16, tag="expn")
                divh = [None] * H
                for h in range(H):
                    tiles = TILES_OF_HEAD[h]
                    ps_s = psSE.tile([SENC, 512], F32, tag="ps")
                    for i, t in enumerate(tiles):
                        nc.tensor.matmul(
                            ps_s,
                            kt_r[t][h % 2][:, bsl],
                            qt[b][:, t, sl],
                            start=(i == 0),
                            stop=(i == len(tiles) - 1),
                        )
                    nc.scalar.activation(
                        out=exps[:, h, :], in_=ps_s, func=AF.Exp, scale=ATTN_SCALE
                    )
                    ps_se = psSE.tile([SENC, 512], F32, tag="ps", name="ps_se")
                    nc.tensor.matmul(
                        ps_se[0:1, :], ones77, exps[:, h, :], start=True, stop=True
                    )
                    nc.vector.reciprocal_approx_fast(
                        out=ps_se[0:1, :], in_=ps_se[0:1, :]
                    )
                    recb = smallp.tile([1, 512], BF16, tag="recb", bufs=1)
                    nc.vector.tensor_copy(out=recb, in_=ps_se[0:1, :])
                    divh[h] = divp.tile([P, 512], BF16, tag="divh", name=f"divh{h}")
                    nc.gpsimd.partition_broadcast(divh[h], recb)
                for t in range(NCO):
                    pairs = PAIRS_OF_TILE[t]
                    ps_av = psV.tile([P, 512], F32, tag="ps")
                    for i, pi in enumerate(pairs):
                        ph = PAIRS[pi][0]
                        nc.tensor.matmul(
                            ps_av,
                            vm[b][:, pi, :],
                            exps[:, ph, :],
                            start=(i == 0),
                            stop=(i == len(pairs) - 1),
                        )
                    # evacuate with fused 1/Z: at = ps_av * divh[head(row)]
                    for pi in pairs:
                        ph, _, lo, hi = PAIRS[pi]
                        for o, sz in aligned_ranges(lo - P * t, hi - P * t):
                            nc.vector.scalar_tensor_tensor(
                                out=at[b][o : o + sz, t, sl],
                                in0=ps_av[o : o + sz, :],
                                scalar=1.0,
                                in1=divh[ph][o : o + sz, :],
                                op0=MULT,
                                op1=MULT,
                            )

        def o_proj(b, wo_s):
            for stile in range(S // P):
                s0 = stile * P
                ost = stag.tile([P, C], BF16, tag="ost")
                pso = [
                    psA.tile([P, 512], F32, tag="ps", name=f"pso{k}")
                    for k in range(len(OCHUNKS))
                ]
                for ci in range(NCI_Q):
                    for k, (c0, cn) in enumerate(OCHUNKS):
                        nc.tensor.matmul(
                            pso[k][:, :cn],
                            at[b][:, ci, s0 : s0 + P],
                            wo_s[:, ci, c0 : c0 + cn],
                            start=(ci == 0),
                            stop=(ci == NCI_Q - 1),
                        )
                for k, (c0, cn) in enumerate(OCHUNKS):
                    nc.vector.tensor_copy(
                        out=ost[:, c0 : c0 + cn], in_=pso[k][:, :cn]
                    )
                nc.sync.dma_start(out=out_d.ap()[b, s0 : s0 + P, :], in_=ost)

        # b-major interleave: attention(b0) overlaps q_proj(b1) on PE;
        # o_proj(b0) overlaps attention(b1).
        q_proj(0)
        v_proj()
        wo_s = wpool.tile([P, NCI_Q, C], BF16, tag="wA", name="wo_s")
        nc.sync.dma_start(
            out=wo_s, in_=wot_d.ap().rearrange("(ci p) c -> p ci c", p=P)
        )
        attention(0)
        q_proj(1)
        o_proj(0, wo_s)
        attention(1)
        o_proj(1, wo_s)

    nc.compile()
    return nc


_NC_CACHE = []


def _get_nc():
    if not _NC_CACHE:
        _NC_CACHE.append(build())
    return _NC_CACHE[0]


def make_in_maps(hidden_states, encoder_hidden_states, Wq, Wk, Wv, Wo,
                 q_down, q_up, k_down, k_up, v_down, v_up, o_down, o_up):
    bf = ml_dtypes.bfloat16
    f8 = ml_dtypes.float8_e4m3
    wq = (Wq.astype(np.float64) + q_up.astype(np.float64) @ q_down.astype(np.float64))
    wk = (Wk.astype(np.float64) + k_up.astype(np.float64) @ k_down.astype(np.float64))
    wv = (Wv.astype(np.float64) + v_up.astype(np.float64) @ v_down.astype(np.float64))
    wo = (Wo.astype(np.float64) + o_up.astype(np.float64) @ o_down.astype(np.float64))
    wqt = np.ascontiguousarray(wq.T).astype(bf)
    wkt = np.ascontiguousarray(wk.T).astype(bf)
    wot = np.ascontiguousarray(wo.T).astype(bf)
    wvt = wv.T  # [CENC, C] f64
    wvm = np.zeros((CENC, NPAIR * P), np.float64)
    for i, (h, t, lo, hi) in enumerate(PAIRS):
        wvm[:, i * P + (lo - P * t) : i * P + (hi - P * t)] = wvt[:, lo:hi]
    wvm = wvm.astype(bf)

    in_maps = []
    for c in range(NCORES):
        hs = hidden_states[c * BPC : (c + 1) * BPC]  # [2, S, C]
        xt = np.ascontiguousarray(hs.transpose(0, 2, 1)).astype(bf)
        enc = encoder_hidden_states[c * BPC : (c + 1) * BPC]  # [2, 77, 1024]
        et = np.empty((CENC, SENC2), np.float32)
        for b in range(BPC):
            et[:, b * SENC : (b + 1) * SENC] = enc[b].T
        in_maps.append(
            {
                "xt": xt,
                "et": et.astype(bf),
                "wqt": wqt,
                "wkt": wkt,
                "wvm": wvm,
                "wot": wot,
            }
        )
    return in_maps


def kernel(hidden_states, encoder_hidden_states, Wq, Wk, Wv, Wo, bo,
           q_down, q_up, k_down, k_up, v_down, v_up, o_down, o_up):
    nc = _get_nc()
    in_maps = make_in_maps(
        hidden_states, encoder_hidden_states, Wq, Wk, Wv, Wo,
        q_down, q_up, k_down, k_up, v_down, v_up, o_down, o_up,
    )
    res = run_bass_kernel_spmd(nc, in_maps, list(range(NCORES)))
    out = np.concatenate(
        [np.asarray(res.results[c]["out"]).astype(np.float32) for c in range(NCORES)],
        axis=0,
    )
    out = out + bo.astype(np.float32)[None, None, :]
    return out.astype(np.float32)
